# revision 33
# baseline (speedup 1.0000x reference)
"""Trainium2 Bass kernel for the deep-hedging Milstein SDE loss.

Math: the reference scan collapses (see derivation in comments below):
  s_{n+1} = s_n * m_n,  m_n = c0 + c1*r_n + c2*r_n^2
  v_{n+1} = v_n + sp_n * (Zw_n - Zu_n^2 * Tf_n)            [per-point phi terms]
where the per-point quantities come from a forward-mode jet of the holding
MLP with THREE streams:
  a  : primal silu chain
  u  : first-order tangent along (0, sqrt(0.5*dt)*SIG*s*r)   [2nd-order probe]
  w  : merged gamma + second-order stream:
         w0 = silu'(z)*Mg + silu''(z)*Mu^2
         w' = silu'(z)*Zw + silu''(z)*Zu^2
(the gamma direction is (dt, Ds); gamma and the 2nd-order stream propagate
with the same linear rule and are only ever used summed, so they merge.)

Layout per core (1024 paths, 128 steps):
  sgrid [128 part = p, 8 blocks b, 128 steps n], path_local = b*128 + p.
  MLP groups g = b // 2 (4 groups of 2 blocks); point column within a group:
      j = x*128 + p,   x = b2*128 + n,  b = 2*g + b2.
  Chunk ci = x in [4ci, 4ci+4) -> 512 columns.

Stage B repack is done on the PE: S5T [p, x, kgp(32: 20 real + 12 pad)]
holds the 5 value planes (t, ones, s, Ds, s*r) interleaved so that one
[128,128] PE transpose per chunk yields the matmul rhs [(x4, kg32), p].
A plain DMA cannot do this repack: the cost model charges per-partition
bytes and the BIR verifier requires the partition-crossing dim first on
both sides, which forbids partition-transposing DMAs.

The w-stream's add (w = q - phi) is folded into the next layer's matmul
via PSUM accumulation with negated weight copies, so only (q, phi) are
materialized. Emission is software-pipelined ("wavefront"): stage j of
chunk k-j is emitted at iteration k, keeping all in-order engine queues
filled with ~6 different chunks' ready work.

Engine split per chunk (V1 CoreSim cost model, ~7.7us each, ~96% busy):
  PE  : 1 transpose + 12 L0 sub-matmuls + 10 hidden + 4 final matmuls
  ACT : Derivative_silu + Tanh + Identity(Zu->f16) per layer
  DVE : sigma (ts 4x), a/q (PSUM tt/stt), P1, copies
  Pool: u, A, D, phi (f16 tensor_tensor, no PSUM access allowed)
"""

import os

import numpy as np

import concourse.bass as bass
import concourse.mybir as mybir
from concourse import tile
from concourse.bass_utils import run_bass_kernel_spmd


# problem constants (hardcoded per spec)
B = 8192
NSTEP = 128
NCORE = 8
BC = B // NCORE          # 1024 paths per core
P = 128                  # partitions
NB = BC // P             # 8 path blocks
WIDTH = 32
NG = 4                   # feature groups on partitions
NH = 3                   # hidden layers
NX = 2 * NSTEP           # 256 x-values (b2, n)
C = NX * P               # 32768 point-columns per group
CC = 512                 # chunk columns (4 x-values * 128 p)
NCHUNK = NX // 4         # 64
KREAL = 20               # 5 value planes * 4 groups
KG = 32                  # padded plane rows per x in S5T
T0, T1 = 0.0, 1.0
MU, SIG = 1.0, 1.0
DT = (T1 - T0) / NSTEP
SQDT = float(np.sqrt(DT))

F32 = mybir.dt.float32
AF = mybir.ActivationFunctionType
ALU = mybir.AluOpType

SD = mybir.dt.float16

_CACHE = {}
DBG_NCHUNK = int(os.environ.get("KDBG_NCHUNK", "0")) or None


def _legalize_waits(nc):
    """Split long on_wait lists into standalone single-wait NoOps.

    This walrus rejects instructions whose sync_info carries more waits
    than the ISA encoding holds. Tile emits up to one wait per logical
    processor, so spill the excess onto NoOps on the same engine queue,
    which execute in order before the real instruction.
    """
    ctr = 0
    for bb in nc.main_func.blocks:
        out = []
        for ins in bb.instructions:
            si = ins.sync_info
            if si is not None and si.on_wait:
                limit = 1
                waits = list(si.on_wait)
                if len(waits) > limit:
                    spill, keep = waits[:-limit], waits[-limit:]
                    for w in spill:
                        ctr += 1
                        nop = mybir.InstNoOp(name=f"waitnop_{ctr}", ins=[], outs=[])
                        nop.engine = ins.engine
                        nop.sync_info = mybir.SyncInfo(on_wait=[w], on_update=[])
                        out.append(nop)
                    si.on_wait = keep
            out.append(ins)
        bb.instructions = out


def _build_program():
    nc = bass.Bass()

    rn_d = nc.declare_dram_parameter("rn_sg", [P, NB * NSTEP], F32, isOutput=False)
    tk_d = nc.declare_dram_parameter("tk", [P, NX * 8], SD, isOutput=False)
    id_d = nc.declare_dram_parameter("ident", [P, P], SD, isOutput=False)
    lhsTL_d = nc.declare_dram_parameter("lhsTL", [12, P, P], SD, isOutput=False)
    lhsTh_d = nc.declare_dram_parameter("lhsTh", [NH, P, P], SD, isOutput=False)
    lhsThN_d = nc.declare_dram_parameter("lhsThN", [NH, P, P], SD, isOutput=False)
    lhsTf_d = nc.declare_dram_parameter("lhsTf", [P, NG], SD, isOutput=False)
    lhsTfN_d = nc.declare_dram_parameter("lhsTfN", [P, NG], SD, isOutput=False)
    bias_d = nc.declare_dram_parameter("bias", [P, 4, 2], F32, isOutput=False)
    bfh_d = nc.declare_dram_parameter("bfh", [P, 1], F32, isOutput=False)
    out_d = nc.declare_dram_parameter("yT", [BC, 2], F32, isOutput=True)

    # m_n = c0 + c1*r + c2*r^2
    c0 = 1.0 + MU * DT - 0.5 * SIG * SIG * DT
    c1 = SIG * SQDT
    c2 = 0.5 * SIG * SIG * DT

    with tile.TileContext(nc) as tc:
        with (
            tc.tile_pool(name="const", bufs=1) as cpool,
            tc.tile_pool(name="sg", bufs=1) as sgpool,
            tc.tile_pool(name="work", bufs=6) as wpool,
            tc.tile_pool(name="stream", bufs=10) as spool,
            tc.tile_pool(name="psum", bufs=7, space="PSUM") as pspool,
            tc.tile_pool(name="pst", bufs=1, space="PSUM") as pstpool,
        ):
            # ---- constants ----
            ident = cpool.tile([P, P], SD, tag="ident")
            lhsTL = [
                cpool.tile([P, P], SD, tag=f"lhsTL{i}", name=f"lhsTL{i}")
                for i in range(12)
            ]
            lhsTh = [
                cpool.tile([P, P], SD, tag=f"lhsTh{l}", name=f"lhsTh{l}")
                for l in range(NH)
            ]
            lhsThN = [
                cpool.tile([P, P], SD, tag=f"lhsThN{l}", name=f"lhsThN{l}")
                for l in range(NH)
            ]
            lhsTf = cpool.tile([P, NG], SD, tag="lhsTf")
            lhsTfN = cpool.tile([P, NG], SD, tag="lhsTfN")
            bias = cpool.tile([P, 4, 2], F32, tag="bias")
            bfh = cpool.tile([P, 1], F32, tag="bfh")
            nc.scalar.dma_start(ident[:], id_d[:])
            for i in range(12):
                nc.scalar.dma_start(lhsTL[i][:], lhsTL_d[i])
            nc.sync.dma_start(bias[:], bias_d[:])

            def bias_r(l, h):
                return bias[:, l, h : h + 1]

            # ---- stage A: sgrid GBM math -> S5T staging ----
            # S5T[p, x, kgp]: kgp = 4k+g; planes k: 0 t, 1 ones, 2 s, 3 Ds, 4 s*r
            S5T = sgpool.tile([P, NX, KG], SD, tag="S5T")
            rs = sgpool.tile([P, NB, NSTEP], F32, tag="rs")
            # constant planes (t, ones) from DRAM
            nc.sync.dma_start(
                S5T[:, :, 0:8],
                tk_d[:].rearrange("p (x k) -> p x k", k=8),
            )
            # pad rows: keep finite for the transpose passthrough
            nc.gpsimd.memset(S5T[:, :, KREAL:KG], 0.0)
            scr = sgpool.tile([P, NB, NSTEP], F32, tag="scr")
            m = sgpool.tile([P, NB, NSTEP], F32, tag="m")
            sfull = sgpool.tile([P, NB, NSTEP + 1], F32, tag="sfull")
            nc.vector.memset(sfull[:, :, 0:1], 1.0)

            # plane views into S5T: iteration (p, g, b2, n) matching sgrid (p, b=2g+b2, n)
            def plane(k):
                return S5T[:].rearrange("p (b2 n) (k g) -> k p g b2 n", k=8, b2=2)[k]

            def sg_gb(t_ap):
                # sgrid [p, b, n] -> [p, g, b2, n]
                return t_ap.rearrange("p (g b2) n -> p g b2 n", g=NG)

            # Stage A split into 4 step-quarters so chunk 0's transpose can
            # start after the first quarter instead of after all of stage A.
            rn_v = rn_d[:].rearrange("p (b n) -> p b n", b=NB)
            NQ = NSTEP // 4
            for qi in range(4):
                ns = slice(qi * NQ, (qi + 1) * NQ)
                nc.sync.dma_start(rs[:, :, ns], rn_v[:, :, ns])
                # m = (c2*r + c1)*r + c0
                nc.vector.tensor_scalar(
                    scr[:, :, ns], rs[:, :, ns], c2, c1, ALU.mult, ALU.add
                )
                nc.vector.tensor_tensor(
                    m[:, :, ns], scr[:, :, ns], rs[:, :, ns], ALU.mult
                )
                nc.vector.tensor_scalar(
                    m[:, :, ns], m[:, :, ns], 1.0, c0, ALU.mult, ALU.add
                )
                for b in range(NB):
                    init = 1.0 if qi == 0 else sfull[:, b, qi * NQ : qi * NQ + 1]
                    nc.vector.tensor_tensor_scan(
                        sfull[:, b, qi * NQ + 1 : (qi + 1) * NQ + 1],
                        m[:, b, ns],
                        m[:, b, ns],
                        init,
                        ALU.mult,
                        ALU.bypass,
                    )
                # s plane (Pool), Ds plane (DVE), s*r plane (Pool); per-b2
                # halves keep the APs <= 3 free dims (walrus limit)
                for b2 in range(2):
                    sNq = sfull[:, b2::2, qi * NQ : (qi + 1) * NQ]
                    mq = m[:, b2::2, ns]
                    rq = rs[:, b2::2, ns]
                    nc.gpsimd.tensor_copy(plane(2)[:, :, b2, ns], sNq)
                    nc.vector.scalar_tensor_tensor(
                        plane(3)[:, :, b2, ns], mq, 1.0, sNq, ALU.subtract, ALU.mult
                    )
                    nc.gpsimd.tensor_tensor(plane(4)[:, :, b2, ns], sNq, rq, ALU.mult)

            # hidden/final weights are needed only once layer-1 runs (~8us in);
            # load them on the otherwise idle SP queue after the rn quarters
            for l in range(NH):
                nc.sync.dma_start(lhsTh[l][:], lhsTh_d[l])
                nc.sync.dma_start(lhsThN[l][:], lhsThN_d[l])
            nc.sync.dma_start(lhsTf[:], lhsTf_d[:])
            nc.sync.dma_start(lhsTfN[:], lhsTfN_d[:])
            nc.sync.dma_start(bfh[:], bfh_d[:])

            # ---- staging for stage D: rows 32s+g, cols j = x*128+p ----
            staging = sgpool.tile([P, C], SD, tag="staging")

            # ---- stage C: chunked MLP jet (wavefront-pipelined emission) ----
            # Stages per chunk c:
            #   j=0: PE transpose + DVE rhsb copy
            #   j=1..4: layer l=j-1: PE matmuls + ACT (s1, T, zu16) + elementwise
            #   j=5: final matmuls + staging copy
            # Emitting stage j of chunk k-j at iteration k keeps every engine
            # queue filled with ~6 different chunks' ready work (in-order
            # engine queues would otherwise stall on the intra-chunk chain).
            nchunk = DBG_NCHUNK or NCHUNK
            cstate = {}

            def st_transpose(c):
                pst = pstpool.tile([P, P], SD, tag="pst")
                nc.tensor.transpose(pst[:], S5T[:, 4 * c : 4 * c + 4, :], ident[:])
                rhsb = spool.tile([P, P], SD, tag="rhsb")
                if c & 1:
                    nc.vector.tensor_copy(rhsb[:], pst[:])
                else:
                    nc.scalar.activation(rhsb[:], pst[:], AF.Identity)
                cstate[c] = {"rhsb": rhsb}

            def st_layer(c, l):
                S = cstate[c]
                if l == 0:
                    Zp = pspool.tile([P, CC], F32, tag="ps")
                    Zw = pspool.tile([P, CC], F32, tag="ps")
                    Zu = pspool.tile([P, CC], F32, tag="ps")
                    rv = S.pop("rhsb")
                    for xi in range(4):
                        sl = slice(xi * P, (xi + 1) * P)
                        nc.tensor.matmul(Zp[:, sl], lhsTL[0 + xi][:], rv[:], start=True, stop=True)
                        nc.tensor.matmul(Zw[:, sl], lhsTL[4 + xi][:], rv[:], start=True, stop=True)
                        nc.tensor.matmul(Zu[:, sl], lhsTL[8 + xi][:], rv[:], start=True, stop=True)
                else:
                    a_p, u_p = S.pop("a"), S.pop("u")
                    q_p, ph_p = S.pop("q"), S.pop("ph")
                    Zp = pspool.tile([P, CC], F32, tag="ps")
                    Zu = pspool.tile([P, CC], F32, tag="ps")
                    Zw = pspool.tile([P, CC], F32, tag="ps")
                    nc.tensor.matmul(Zp[:], lhsTh[l - 1][:], a_p[:], start=True, stop=True)
                    nc.tensor.matmul(Zu[:], lhsTh[l - 1][:], u_p[:], start=True, stop=True)
                    # w = q - ph folded into the matmul: Zw = W*q + (-W)*ph
                    nc.tensor.matmul(Zw[:], lhsTh[l - 1][:], q_p[:], start=True, stop=False)
                    nc.tensor.matmul(Zw[:], lhsThN[l - 1][:], ph_p[:], start=False, stop=True)

                s1 = wpool.tile([P, CC], SD, tag="s1")
                nc.scalar.activation(
                    s1[:], Zp[:], AF.Derivative_silu, bias=bias_r(l, 0)
                )
                T = wpool.tile([P, CC], SD, tag="T")
                nc.scalar.activation(
                    T[:], Zp[:], AF.Tanh, bias=bias_r(l, 1), scale=0.5
                )
                zu16 = wpool.tile([P, CC], SD, tag="zu16")
                nc.scalar.activation(zu16[:], Zu[:], AF.Identity)

                # sigma = 0.5*T + 0.5                        (DVE ts 4x)
                sg = wpool.tile([P, CC], SD, tag="sg")
                nc.vector.tensor_scalar(sg[:], T[:], 0.5, 0.5, ALU.mult, ALU.add)
                # a' = (Zp + b) * sigma                      (DVE stt, PSUM)
                a = spool.tile([P, CC], SD, tag="a")
                nc.vector.scalar_tensor_tensor(
                    a[:], Zp[:], bias_r(l, 0), sg[:], ALU.add, ALU.mult
                )
                # q = Zw * s1                                (DVE tt, PSUM)
                q = spool.tile([P, CC], SD, tag="q")
                nc.vector.tensor_tensor(q[:], Zw[:], s1[:], ALU.mult)
                # u' = zu16 * s1                             (Pool)
                u = spool.tile([P, CC], SD, tag="u")
                nc.gpsimd.tensor_tensor(u[:], zu16[:], s1[:], ALU.mult)
                # A = zu16^2                                 (Pool)
                A = wpool.tile([P, CC], SD, tag="A")
                nc.gpsimd.tensor_tensor(A[:], zu16[:], zu16[:], ALU.mult)
                # P1 = s1*T; D = P1 - sigma = -silu''
                P1 = wpool.tile([P, CC], SD, tag="P1")
                peng = nc.vector if (l < 2 or (l == 2 and c % 4 == 0)) else nc.gpsimd
                peng.tensor_tensor(P1[:], s1[:], T[:], ALU.mult)
                D = wpool.tile([P, CC], SD, tag="D")
                nc.gpsimd.tensor_tensor(D[:], P1[:], sg[:], ALU.subtract)
                # phi = D*A = -silu''*A  (w' = q - phi folds into next matmul)
                ph = spool.tile([P, CC], SD, tag="ph")
                nc.gpsimd.tensor_tensor(ph[:], D[:], A[:], ALU.mult)
                S["a"], S["u"], S["q"], S["ph"] = a, u, q, ph

            def st_final(c):
                S = cstate.pop(c)
                a, u, q, ph = S["a"], S["u"], S["q"], S["ph"]
                F = pspool.tile([P, CC], F32, tag="ps")
                nc.tensor.matmul(F[0:4, :], lhsTf[:], a[:], start=True, stop=True)
                nc.tensor.matmul(F[32:36, :], lhsTf[:], u[:], start=True, stop=True)
                nc.tensor.matmul(F[64:68, :], lhsTf[:], q[:], start=True, stop=False)
                nc.tensor.matmul(F[64:68, :], lhsTfN[:], ph[:], start=False, stop=True)
                dst = staging[0:68, :].rearrange("r (p x) -> r p x", p=P)[
                    :, :, 4 * c : 4 * c + 4
                ]
                fin = F[0:68, :].rearrange("r (xi p) -> r p xi", xi=4)
                nc.vector.tensor_copy(dst, fin)

            for k in range(nchunk + 6):
                for j in range(6):
                    c = k - j
                    if not (0 <= c < nchunk):
                        continue
                    if j == 0:
                        st_transpose(c)
                    elif j < 5:
                        st_layer(c, j - 1)
                    else:
                        st_final(c)

            # ---- stage D: repack + phi assembly + reduction ----
            zf_sg = sgpool.tile([P, NB, NSTEP], SD, tag="zf_sg")
            uf_sg = sgpool.tile([P, NB, NSTEP], SD, tag="uf_sg")
            wf_sg = sgpool.tile([P, NB, NSTEP], SD, tag="wf_sg")
            if DBG_NCHUNK is None:
                qeng = (nc.sync, nc.sync, nc.gpsimd)
                for s, sgt in enumerate((zf_sg, uf_sg, wf_sg)):
                    for g in range(NG):
                        row = 32 * s + g
                        src = staging[row : row + 1, :].rearrange(
                            "one (p x) -> one p x", p=P
                        )
                        dst = sgt[:, 2 * g : 2 * g + 2, :].rearrange(
                            "p b2 n -> p (b2 n)"
                        )
                        qeng[s].dma_start(dst, src)

                Tf = sgpool.tile([P, NB, NSTEP], SD, tag="Tf")
                nc.scalar.activation(Tf[:], zf_sg[:], AF.Tanh, bias=bfh[:], scale=0.5)
                U2 = sgpool.tile([P, NB, NSTEP], SD, tag="U2")
                nc.gpsimd.tensor_tensor(U2[:], uf_sg[:], uf_sg[:], ALU.mult)
                T2 = sgpool.tile([P, NB, NSTEP], SD, tag="T2")
                nc.vector.tensor_tensor(T2[:], Tf[:], Tf[:], ALU.mult)
                Q = sgpool.tile([P, NB, NSTEP], SD, tag="Q")
                nc.gpsimd.tensor_tensor(Q[:], U2[:], Tf[:], ALU.mult)
                sp = sgpool.tile([P, NB, NSTEP], SD, tag="sp")
                nc.vector.tensor_scalar(sp[:], T2[:], -0.25, 0.25, ALU.mult, ALU.add)
                Dd = sgpool.tile([P, NB, NSTEP], SD, tag="Dd")
                nc.vector.tensor_tensor(Dd[:], wf_sg[:], Q[:], ALU.subtract)
                Sd = sgpool.tile([P, NB, NSTEP], SD, tag="Sd")
                vT = sgpool.tile([P, NB], F32, tag="vT")
                for b in range(NB):
                    nc.vector.scalar_tensor_tensor(
                        Sd[:, b, :],
                        Dd[:, b, :],
                        0.0,
                        sp[:, b, :],
                        ALU.add,
                        ALU.mult,
                        accum_out=vT[:, b : b + 1],
                    )
            else:
                vT = sgpool.tile([P, NB], F32, tag="vT")
                nc.vector.memset(vT[:], 0.0)

            # ---- outputs ----
            yv = out_d[:].rearrange("(b p) c -> p b c", p=P)
            nc.sync.dma_start(yv[:, :, 0:1], sfull[:, :, NSTEP : NSTEP + 1])
            nc.sync.dma_start(
                yv[:, :, 1:2], vT[:].rearrange("p (b one) -> p b one", one=1)
            )

    _legalize_waits(nc)
    return nc


def _prep_host(inputs):
    rnorm = np.ascontiguousarray(np.asarray(inputs["rnorm"], dtype=np.float32))
    W0 = np.asarray(inputs["W0"], dtype=np.float32)
    b0 = np.asarray(inputs["b0"], dtype=np.float32)
    Wh = np.asarray(inputs["Wh"], dtype=np.float32)
    bh = np.asarray(inputs["bh"], dtype=np.float32)
    Wf = np.asarray(inputs["Wf"], dtype=np.float32)
    bf = np.asarray(inputs["bf"], dtype=np.float32)

    sd_np = mybir.dt.np(SD)

    # tk[p, x*8 + kgp]: t/ones planes; t = DT*(x mod 128) for every p
    tk = np.zeros((P, NX, 8), np.float32)
    n_of_x = np.tile(np.arange(NSTEP, dtype=np.float32), 2)
    for g in range(NG):
        tk[:, :, 0 + g] = (DT * n_of_x)[None, :]
        tk[:, :, 4 + g] = 1.0

    ident = np.eye(P, dtype=np.float32)

    # lhsT rows r = 4k+g; planes k: 0 t, 1 ones, 2 s, 3 Ds, 4 s*r
    # K=128 with 4 quadrant-masked variants per stream: variant xi has the
    # 32-row coef block at partitions 32*xi and zeros elsewhere
    l0 = np.zeros((KG, P), np.float32)
    lg = np.zeros((KG, P), np.float32)
    lu = np.zeros((KG, P), np.float32)
    for g in range(NG):
        cols = slice(32 * g, 32 * (g + 1))
        l0[4 * 0 + g, cols] = W0[:, 0]                          # t
        l0[4 * 2 + g, cols] = W0[:, 1]                          # s
        lg[4 * 1 + g, cols] = W0[:, 0] * DT                     # ones -> dhdt*dt
        lg[4 * 3 + g, cols] = W0[:, 1]                          # Ds
        lu[4 * 4 + g, cols] = W0[:, 1] * SIG * float(np.sqrt(0.5 * DT))
    lhsTL = np.zeros((12, P, P), np.float32)
    for s, blk in enumerate((l0, lg, lu)):
        for xi in range(4):
            lhsTL[s * 4 + xi, 32 * xi : 32 * (xi + 1), :] = blk

    lhsTh = np.zeros((NH, P, P), np.float32)
    for l in range(NH):
        for g in range(NG):
            blk = slice(32 * g, 32 * (g + 1))
            lhsTh[l, blk, blk] = Wh[l].T
    lhsTf = np.zeros((P, NG), np.float32)
    for g in range(NG):
        lhsTf[32 * g : 32 * (g + 1), g] = Wf[0]
    lhsThN = -lhsTh
    lhsTfN = -lhsTf

    bias = np.zeros((P, 4, 2), np.float32)
    bias[:, 0, 0] = np.tile(b0, NG)
    bias[:, 0, 1] = 0.5 * bias[:, 0, 0]
    for l in range(NH):
        bias[:, l + 1, 0] = np.tile(bh[l], NG)
        bias[:, l + 1, 1] = 0.5 * bias[:, l + 1, 0]
    bfh = np.full((P, 1), 0.5 * bf[0], np.float32)

    shared = {
        "tk": tk.reshape(P, NX * 8).astype(sd_np),
        "ident": ident.astype(sd_np),
        "lhsTL": lhsTL.astype(sd_np),
        "lhsTh": lhsTh.astype(sd_np),
        "lhsThN": lhsThN.astype(sd_np),
        "lhsTf": lhsTf.astype(sd_np),
        "lhsTfN": lhsTfN.astype(sd_np),
        "bias": bias,
        "bfh": bfh,
    }

    in_maps = []
    for core in range(NCORE):
        shard = rnorm[core * BC : (core + 1) * BC]          # [1024, 128]
        sg = np.ascontiguousarray(
            shard.reshape(NB, P, NSTEP).transpose(1, 0, 2).reshape(P, NB * NSTEP)
        )
        in_maps.append({"rn_sg": sg, **shared})
    return in_maps


last_perf = {}


def kernel(trace=False, **inputs) -> np.ndarray:
    if "nc" not in _CACHE:
        _CACHE["nc"] = _build_program()
    nc = _CACHE["nc"]
    in_maps = _prep_host(inputs)
    res = run_bass_kernel_spmd(nc, in_maps, list(range(NCORE)), trace=trace)
    last_perf["exec_time_ns"] = res.exec_time_ns
    out = np.empty((B, 2), np.float32)
    for core in range(NCORE):
        yt = res.results[core]["yT"]                        # [1024, 2]
        out[core * BC : (core + 1) * BC] = yt
    return out
